# revision 60
# baseline (speedup 1.0000x reference)
"""Trainium2 Bass kernel for nn_Attention_22539988369511.

Dense transformer attention block (B=4, N=2048, C=1024, H=16, hd=64),
sharded over 8 NeuronCores with tensor parallelism over heads (2 heads
per core), AllToAll to re-shard from heads to tokens before the output
projection, host concatenation of per-core token slices.

All-bf16 compute (fp8 was measured: in this diffuse-attention regime any
fp8 tensor in the attention path adds ~3.5% output error - over budget).

v8 design (668us -> ~545us):
 - scores via per-head BLOCK-DIAGONAL stationaries: kdiag[h] holds the
   head's k on partition rows 0:64 at key-columns c%128<64 and on rows
   64:128 at c%128>=64 (zeros elsewhere); moving data qdup[h] carries the
   head's q duplicated on both partition halves. One [128,128]x[128,512]
   matmul then yields all 128 keys' scores - 2 matmuls per key-tile
   instead of 4, halving the PE hold time of the scores stage (the PE
   bills by moving columns, so the old two-quadrant scheme paid 2x).
 - attention loops qc-outer/hl-inner; each batch reshards in two
   half-token AllToAlls. Each chunk's denominator epilogue (DVE recip +
   1/den broadcast) is pipelined into the NEXT chunk's first kt slots so
   its PE piece never sits at the queue head, and the reshard fires from
   inside that epilogue. PV accumulates in per-qh [65,512] psum tiles
   from a 3-deep pool so consecutive chunks never serialize on psum.
 - proj uses only the psml psum pool (never the scores pool - a gat DMA
   stall must not couple into the exp chain), bias is folded into the
   psum->sbuf copy via a preloaded broadcast tile, and the last three
   projection half-chunks execute inside the final collective's ~16us
   latency window instead of eating attention-time PE slack.
 - a2a staging DMAs were measured cheap on Sync; DMAs must NEVER ride
   the Scalar queue (they stall the exp pacer) and collectives hold the
   GpSimd SEQ while waiting, so nothing latency-critical goes there.
 - v5: w_q/w_k/x-chunk DMAs split for an earlier first matmul (a write
   must be EMITTED before any instruction that reads it - the tile
   framework tracks writers at emission time); projection half-chunks
   load-balanced across batch filler windows (b3 has no qkv filler and
   is ACT-paced, so it absorbs extra proj work) with three more riding
   the final collective's latency window. XBAR dma_start_transpose for
   the v-store produced garbage and a norm(1)-inside-attention prologue
   cost +75us (norm Ln/Exp interleave into the exp chain) - both
   reverted.
 - v6: filler bursts smoothed (one qkv accumulation chain per filler
   slot, proj chains split mid-way) so no single slot dumps >2us of PE
   work ahead of the next kt's scores in the in-order queue; pt pool
   deepened to 5 bufs (funded by nrm pool 2->1) so the exp chain rides
   ahead of PV consumption; PV lagged two key-tiles so both it and the
   scores-psum reuse gate on exp(kt-2), not exp(kt-1). att bufs beyond 5
   are useless (exp lookahead is capped by the 2 scores psum buffers)
   and staging bufs=1 serializes the qkv pipeline - both measured
   worse.
 - v7: startup loads fully streamed (first x half-chunk, then wq per
   c-chunk interleaved with the second x half) so the first q matmuls
   begin when ~1/8 of the weights have landed; tail projections run
   their second chain on the scores psum pool (attention is over, no
   coupling risk), de-serializing the single psml buffer.
 - v8: psum banks rebalanced pacc 3->2 / psml 1->2 - the epilogue
   pipelining already drains each chunk's pv reads in the next chunk's
   first slots, so 2 pv buffers suffice, and the second psml buffer
   de-serializes the v-matmul/norm/proj small-psum rotation inside the
   filler windows.
"""
import os
import sys

import numpy as np
import ml_dtypes

for _p in ("/opt/trn_rl_repo", "/root/.axon_site/_ro/trn_rl_repo"):
    if os.path.isdir(_p) and _p not in sys.path:
        sys.path.append(_p)

import concourse.bass as bass
import concourse.mybir as mybir
from concourse import bacc, tile
from concourse.bass_utils import run_bass_kernel_spmd

# Steer the act-table chooser to the single table that holds BOTH Ln and
# Exp so the kernel runs with zero ACT_TABLE_LOAD swaps (~31us otherwise).
# Order and length MUST be preserved (dict position is the
# act_func_set_id walrus resolves against act_info.json); only the
# membership sets are edited so Exp resolves solely to the ln+exp table.
_orig_get_tables = bacc.get_activation_tables


def _tables_exp_pinned(arch):
    tabs = _orig_get_tables(arch)
    out = {}
    for name, funcs in tabs.items():
        if name != "natural_log_exp_and_others":
            funcs = funcs - {mybir.ActivationFunctionType.Exp}
        out[name] = funcs
    return out


bacc.get_activation_tables = _tables_exp_pinned


BF16 = ml_dtypes.bfloat16
F32 = mybir.dt.float32
BF = mybir.dt.bfloat16
EXP = mybir.ActivationFunctionType.Exp
LN = mybir.ActivationFunctionType.Ln

NCORE = 8
B, N, C, H, HD = 4, 2048, 1024, 16, 64
T = B * N                 # 8192 tokens
HL = H // NCORE           # 2 heads per core
QKCH = HL * HD            # 128 q (or k) channels per core
TSLICE = T // NCORE       # 1024 tokens per core for the output projection
TOKC = 1024               # qkv/norm token chunk
QC = 1024                 # attention q chunk
KT = N // 128             # 128-key tiles per batch (16)
NH = N // 2               # token half for the per-half reshard (1024)
WTOK = NH // NCORE        # 128 tokens per core per half-reshard
LN_BIAS = float(HD) * 1e-6

_BUILD_CACHE = {}


def _build():
    if "nc" in _BUILD_CACHE:
        return _BUILD_CACHE["nc"]
    nc = bacc.Bacc(None, target_bir_lowering=False, debug=True)

    xT_d = nc.declare_dram_parameter("xT", [C, T], BF, isOutput=False)
    wqkvT_d = nc.declare_dram_parameter("wqkvT", [C, 3 * QKCH], BF, isOutput=False)
    wpT_d = nc.declare_dram_parameter("wpT", [C, C], BF, isOutput=False)
    bp_d = nc.declare_dram_parameter("bp", [1, C], BF, isOutput=False)
    cosq_d = nc.declare_dram_parameter("cosq", [HD, N], BF, isOutput=False)
    sinq_d = nc.declare_dram_parameter("sinq", [HD, N], BF, isOutput=False)
    cosk_d = nc.declare_dram_parameter("cosk", [HD, N], BF, isOutput=False)
    sink_d = nc.declare_dram_parameter("sink", [HD, N], BF, isOutput=False)
    # consts: [:, 0:128] = ones2 (row h = ones on cols 64h:64h+64),
    # [:, 128:130] = diag(8)
    consts_d = nc.declare_dram_parameter("consts", [2, 130], BF, isOutput=False)
    out_d = nc.declare_dram_parameter("out", [TSLICE, C], mybir.dt.float32, isOutput=True)

    # re-shard per half-batch: collective (b,h) ships BOTH heads' channels
    # for tokens b*N + h*NH : +NH; core j receives a 128-token sub-slice.
    a2a_in = [[nc.dram_tensor(f"a2a_in{b}_{h}", [NCORE, QKCH, WTOK], BF)
               for h in range(2)] for b in range(B)]
    a2a_out = [[nc.dram_tensor(f"a2a_out{b}_{h}", [NCORE, QKCH, WTOK], BF)
                for h in range(2)] for b in range(B)]

    h2 = HD // 2

    with tile.TileContext(nc) as tc:
        with (
            tc.tile_pool(name="persist", bufs=1) as pp,
            tc.tile_pool(name="xt", bufs=2) as xtp,
            tc.tile_pool(name="nrm", bufs=1) as nrm,
            tc.tile_pool(name="nrm1", bufs=2) as nrm1,
            tc.tile_pool(name="qstg", bufs=2) as qstg,
            tc.tile_pool(name="kstg", bufs=2) as kstg,
            tc.tile_pool(name="ln1", bufs=1) as ln1,
            tc.tile_pool(name="att", bufs=5) as att,
            tc.tile_pool(name="attn1", bufs=1) as attn1,
            tc.tile_pool(name="wpp", bufs=1) as wpp,
            tc.tile_pool(name="gatp", bufs=2) as gatp,
            tc.tile_pool(name="yp", bufs=1) as yp,
            tc.tile_pool(name="pbig", bufs=2, space="PSUM") as pbig,
            tc.tile_pool(name="pacc", bufs=2, space="PSUM") as pacc,
            tc.tile_pool(name="psml", bufs=2, space="PSUM") as psml,
        ):
            # ---- resident tiles ----
            w_sb = pp.tile([128, 8, 3 * QKCH], BF)      # wqkvT, c-tiled
            bp_sb = pp.tile([1, C], BF)
            # rope tables duplicated on partitions 64:128 so both heads'
            # rows can be multiplied in one op
            rope_sb = pp.tile([128, 4, N], BF)          # cosq|sinq|cosk|sink
            # per-chunk staging for q/k between the qkv matmuls and the
            # norm/rope pipeline (full-T stores are no longer needed: the
            # attention reads only qdup/kdiag)
            stg = {}
            # block-diag k and duplicated q per head for 1-matmul scores
            kdiag0 = pp.tile([128, T], BF)
            kdiag1 = pp.tile([128, T], BF)
            qdup0 = pp.tile([128, T], BF)
            qdup1 = pp.tile([128, T], BF)
            kdiag = [kdiag0, kdiag1]
            qdup = [qdup0, qdup1]
            vstore = pp.tile([128, T // 128, 2 * (HD + 1)], BF)
            shard = pp.tile([QKCH, T], BF)              # normalized out^T shard
            rkcol = pp.tile([128, HL, T // 128], F32)   # 8*r_k, column layout
            ones2_sb = pp.tile([2, 128], BF)            # r broadcast stationary
            diag8_sb = pp.tile([2, 2], BF)              # r_k transpose rhs
            ones128_2 = pp.tile([128, 2], BF)           # ss stationary
            ones1_64 = pp.tile([1, HD], BF)
            ones1_128 = pp.tile([1, 128], BF)
            epsb = pp.tile([128, 1], F32)               # ln bias row
            bpb = pp.tile([128, C], BF)                 # bias broadcast rows

            wqr = wqkvT_d[:].rearrange("(c p) n -> p c n", p=128)

            def emit_rope_loads():
                for i, td in enumerate((cosq_d, sinq_d, cosk_d, sink_d)):
                    nc.sync.dma_start(rope_sb[0:HD, i, :], td[:])
                    nc.sync.dma_start(rope_sb[HD:128, i, :], td[:])

            def emit_setup():
                nc.sync.dma_start(bp_sb[:], bp_d[:])
                nc.sync.dma_start(ones2_sb[:], consts_d[:, 0:128])
                nc.sync.dma_start(diag8_sb[:], consts_d[:, 128:130])

            def emit_bpb():
                # broadcast b_proj across partitions once; proj then folds the
                # bias into its psum->sbuf copy instead of a PE matmul
                for d5 in range(2):
                    psb0 = psml.tile([128, 512], F32, tag="psml")
                    nc.tensor.matmul(
                        psb0[:], ones1_128[:], bp_sb[:, d5 * 512:(d5 + 1) * 512],
                        start=True, stop=True)
                    nc.vector.tensor_copy(bpb[:, d5 * 512:(d5 + 1) * 512], psb0[:])

            # zero the kdiag off-blocks once (positions never change roles)
            nc.vector.memset(kdiag[0][:], 0.0)
            nc.gpsimd.memset(kdiag[1][:], 0.0)
            nc.vector.memset(ones128_2[:], 0.0)
            nc.vector.memset(ones128_2[0:64, 0:1], 1.0)
            nc.vector.memset(ones128_2[64:128, 1:2], 1.0)
            nc.vector.memset(ones1_64[:], 1.0)
            nc.vector.memset(ones1_128[:], 1.0)
            nc.vector.memset(epsb[:], LN_BIAS)
            nc.gpsimd.memset(vstore[:, :, HD:HD + 1], 1.0)
            nc.gpsimd.memset(vstore[:, :, 2 * HD + 1:2 * HD + 2], 1.0)

            def qkv_steps(ti):
                tok0 = ti * TOKC
                qs = qstg.tile([QKCH, TOKC], BF, tag="qstg")
                ks = kstg.tile([QKCH, TOKC], BF, tag="kstg")
                stg[ti] = (qs, ks)
                for t5 in range(2):
                    tk0 = tok0 + t5 * 512
                    if (ti, t5) in pre_xt:
                        xt = pre_xt.pop((ti, t5))
                    else:
                        xt = xtp.tile([128, 8, 512], BF, tag="xt")
                        xr = xT_d[:].rearrange("(c p) t -> p c t", p=128)
                        nc.sync.dma_start(
                            xt[:, 0:4, :], xr[:, 0:4, tk0:tk0 + 512])
                        nc.sync.dma_start(
                            xt[:, 4:8, :], xr[:, 4:8, tk0:tk0 + 512])
                    # q and k chains share one pbig tile; each 512-wide half
                    # is its own psum bank (zero-region safe)
                    pqk = pbig.tile([128, 1024], F32, tag="pbig")
                    for m, store in ((0, qs), (1, ks)):
                        ps = pqk[:, m * 512:(m + 1) * 512]
                        for c in range(8):
                            nc.tensor.matmul(
                                ps,
                                w_sb[:, c, m * QKCH:(m + 1) * QKCH],
                                xt[:, c, :],
                                start=(c == 0), stop=(c == 7))
                        nc.vector.tensor_copy(
                            store[:, t5 * 512:(t5 + 1) * 512], ps)
                        # one chain per filler slot: a 3.4us PE burst in a
                        # single slot head-of-line-blocks the next kt's
                        # scores and stretches the exp chain
                        yield
                    for t1 in range(4):
                        psv = psml.tile([128, 512], F32, tag="psml")
                        for c in range(8):
                            nc.tensor.matmul(
                                psv[:, 0:128],
                                xt[:, c, t1 * 128:(t1 + 1) * 128],
                                w_sb[:, c, 2 * QKCH:3 * QKCH],
                                start=(c == 0), stop=(c == 7))
                        g = (tk0 // 128) + t1
                        nc.vector.tensor_copy(
                            vstore[:, g, :].rearrange(
                                "p (a b) -> p a b", b=HD + 1)[:, :, 0:HD],
                            psv[:, 0:128].rearrange("p (a b) -> p a b", b=HD))
                        if t1 == 1:
                            yield
                    yield

            def blk(ap, off):
                """[64, ntok] col-slice -> [64, ntok/128, 64] at +off in each
                128-col block."""
                return ap.rearrange("p (b c) -> p b c", c=128)[:, :, off:off + 64]

            def norm_steps(ti):
                tok0 = ti * TOKC
                n0 = tok0 % N
                qs, ks = stg.pop(ti)
                stores = ((0, qs), (1, ks))
                # ss per head -> r = exp(-0.5*ln(ss + 64eps)) = 1/(8*rms).
                # All Ln ops cluster, then all Exp ops, to minimize
                # activation-table swaps on the Scalar engine.
                lnouts = {}
                for m, store in stores:
                    slf = store[:, :]
                    sq = nrm.tile([128, TOKC], BF, tag="sq")
                    nc.vector.tensor_mul(sq[:], slf, slf)
                    lnout = ln1.tile([2, TOKC], F32,
                                     tag="lnq" if m == 0 else "lnk")
                    lnouts[m] = lnout
                    for h5 in range(2):
                        ps = psml.tile([128, 512], F32, tag="psml")
                        nc.tensor.matmul(
                            ps[0:2, :], ones128_2[:],
                            sq[:, h5 * 512:(h5 + 1) * 512],
                            start=True, stop=True)
                        nc.scalar.activation(
                            lnout[:, h5 * 512:(h5 + 1) * 512], ps[0:2, :],
                            LN, bias=epsb[0:2, 0:1], scale=1.0)
                    yield
                rr2s = {}
                for m, _ in stores:
                    rr2 = nrm1.tile([2, TOKC], BF, tag="rr2q" if m == 0 else "rr2k")
                    rr2s[m] = rr2
                    nc.scalar.activation(rr2[:], lnouts[m][:], EXP, scale=-0.5)
                for m, store in stores:
                    slf = store[:, :]
                    rr2 = rr2s[m]
                    # rope (tables carry q/k_norm_w and rotate-half signs)
                    qrot = nrm.tile([128, TOKC], BF, tag="qrot")
                    for r0 in (0, HD):
                        nc.vector.tensor_copy(
                            qrot[r0:r0 + h2, :],
                            store[r0 + h2:r0 + HD, :])
                        nc.vector.tensor_copy(
                            qrot[r0 + h2:r0 + HD, :],
                            store[r0:r0 + h2, :])
                    cw = rope_sb[:, 2 * m, n0:n0 + TOKC]
                    sw = rope_sb[:, 2 * m + 1, n0:n0 + TOKC]
                    nc.vector.tensor_mul(slf, slf, cw)
                    nc.vector.tensor_mul(qrot[:], qrot[:], sw)
                    nc.vector.tensor_add(slf, slf, qrot[:])
                    yield
                    if m == 0:
                        # q: r broadcast via one K=2 matmul, then scale the
                        # two head rows straight into their qdup halves
                        for h5 in range(2):
                            psb = psml.tile([128, 512], F32, tag="psml")
                            nc.tensor.matmul(
                                psb[:], ones2_sb[:],
                                rr2[:, h5 * 512:(h5 + 1) * 512],
                                start=True, stop=True)
                            c0 = tok0 + h5 * 512
                            nc.vector.tensor_mul(
                                qdup[0][0:HD, c0:c0 + 512],
                                slf[0:HD, h5 * 512:(h5 + 1) * 512], psb[0:HD, :])
                            nc.vector.tensor_mul(
                                qdup[1][HD:128, c0:c0 + 512],
                                slf[HD:128, h5 * 512:(h5 + 1) * 512],
                                psb[HD:128, :])
                        # duplicate each head's q onto the other partition
                        # half (cross-partition -> DMA; engines idle here)
                        nc.sync.dma_start(
                            qdup[0][HD:128, tok0:tok0 + TOKC],
                            qdup[0][0:HD, tok0:tok0 + TOKC])
                        nc.sync.dma_start(
                            qdup[1][0:HD, tok0:tok0 + TOKC],
                            qdup[1][HD:128, tok0:tok0 + TOKC])
                        yield
                    else:
                        # k: 8*r_k rides the exp scale; transpose both heads
                        # into column layout via K=2 matmuls against diag(8)
                        for g in range(TOKC // 128):
                            pst = psml.tile([128, 512], F32, tag="psml")
                            nc.tensor.matmul(
                                pst[:, 0:2],
                                rr2[:, g * 128:(g + 1) * 128],
                                diag8_sb[:],
                                start=True, stop=True)
                            gg = tok0 // 128 + g
                            nc.vector.tensor_copy(
                                rkcol[:, :, gg:gg + 1],
                                pst[:, 0:2].rearrange("p (a b) -> p a b", b=1))
                        yield
                        # scatter k into the block-diagonal stores: head h
                        # rows at key-cols c%128<64 on partitions 0:64 and
                        # c%128>=64 on 64:128 (same-partition halves on DVE,
                        # cross-partition halves on DMA)
                        kc = slf  # kstore slice, rope'd k
                        dst = kdiag[0][0:HD, tok0:tok0 + TOKC]
                        nc.vector.tensor_copy(blk(dst, 0), blk(kc[0:HD, :], 0))
                        dst = kdiag[1][HD:128, tok0:tok0 + TOKC]
                        nc.vector.tensor_copy(
                            blk(dst, 64), blk(kc[HD:128, :], 64))
                        dst = kdiag[0][HD:128, tok0:tok0 + TOKC]
                        nc.sync.dma_start(blk(dst, 64), blk(kc[0:HD, :], 64))
                        dst = kdiag[1][0:HD, tok0:tok0 + TOKC]
                        nc.sync.dma_start(blk(dst, 0), blk(kc[HD:128, :], 0))
                        yield

            def attention_chunk(hl, b, qc, filler=None, prev_epi=None):
                boff = b * N
                qoff = boff + qc * QC
                pv0 = pacc.tile([HD + 1, 512], F32, tag="pacc")
                pv1 = pacc.tile([HD + 1, 512], F32, tag="pacc")
                pvh = [pv0, pv1]

                def emit_pv(pkt, ppt, stop):
                    for qh in range(QC // 512):
                        nc.tensor.matmul(
                            pvh[qh][:],
                            vstore[:, (boff // 128) + pkt,
                                   (HD + 1) * hl:(HD + 1) * (hl + 1)],
                            ppt[:, qh * 512:(qh + 1) * 512],
                            start=(pkt == 0), stop=stop)

                pts = {}
                for kt in range(KT):
                    koff = boff + kt * 128
                    sps = pbig.tile([128, QC], F32, tag="pbig")
                    for qh in range(QC // 512):
                        q0 = qoff + qh * 512
                        nc.tensor.matmul(
                            sps[:, qh * 512:(qh + 1) * 512],
                            kdiag[hl][:, koff:koff + 128],
                            qdup[hl][:, q0:q0 + 512],
                            start=True, stop=True)
                    # PV lags TWO tiles: both this PV and the scores-psum
                    # reuse then gate on exp(kt-2) rather than exp(kt-1),
                    # giving the in-order PE queue a full extra exp period
                    # of slack before it can stall on the ACT chain
                    if kt >= 2:
                        emit_pv(kt - 2, pts.pop(kt - 2), stop=False)
                    pt = att.tile([128, QC], BF, tag="pt")
                    nc.scalar.activation(
                        pt[:], sps[:], EXP,
                        scale=rkcol[:, hl, koff // 128:koff // 128 + 1])
                    pts[kt] = pt
                    # the previous chunk's denominator epilogue rides the
                    # first kt slots so its PE work (the 1/den broadcast)
                    # never sits at the queue head blocking this chunk's
                    # scores while DVE computes the reciprocals
                    if prev_epi is not None:
                        next(prev_epi, None)
                    if filler is not None:
                        next(filler, None)
                emit_pv(KT - 2, pts.pop(KT - 2), stop=False)
                emit_pv(KT - 1, pts.pop(KT - 1), stop=True)
                return pvh

            def chunk_epilogue(hl, b, qc, pvh, do_reshard):
                # denominator -> reciprocal -> scale into shard
                r0 = HD * hl
                qoff = b * N + qc * QC
                drecb = attn1.tile([1, QC], BF, tag="drecb")
                for q5 in range(QC // 512):
                    den0 = attn1.tile([1, 512], F32, tag="den0")
                    nc.vector.tensor_copy(
                        den0[:], pvh[q5][HD:HD + 1, :])
                    drec = attn1.tile([1, 512], F32, tag="drec")
                    with nc.allow_low_precision(reason="softmax denom"):
                        nc.vector.reciprocal_approx_fast(drec[:], den0[:])
                    nc.vector.tensor_copy(
                        drecb[:, q5 * 512:(q5 + 1) * 512], drec[:])
                yield
                pvs = attn1.tile([HD, QC], BF, tag="pvs")
                for q5 in range(QC // 512):
                    nc.vector.tensor_copy(
                        pvs[:, q5 * 512:(q5 + 1) * 512], pvh[q5][0:HD, :])
                yield
                base = b * N + qc * NH
                for q5 in range(QC // 512):
                    dbc = psml.tile([128, 512], F32, tag="psml")
                    nc.tensor.matmul(
                        dbc[0:HD, :], ones1_64[:],
                        drecb[:, q5 * 512:(q5 + 1) * 512],
                        start=True, stop=True)
                    nc.vector.tensor_mul(
                        shard[r0:r0 + HD,
                              qoff + q5 * 512:qoff + (q5 + 1) * 512],
                        pvs[:, q5 * 512:(q5 + 1) * 512], dbc[0:HD, :])
                    if do_reshard:
                        # stage each half as soon as its scale-out lands so
                        # the collective launches ~1us earlier
                        for j in range(q5 * 4, q5 * 4 + 4):
                            nc.sync.dma_start(
                                a2a_in[b][qc][j],
                                shard[:, base + WTOK * j:base + WTOK * (j + 1)])
                yield
                if do_reshard:
                    nc.gpsimd.collective_compute(
                        "AllToAll",
                        mybir.AluOpType.bypass,
                        replica_groups=[list(range(NCORE))],
                        ins=[a2a_in[b][qc][:]],
                        outs=[a2a_out[b][qc][:]],
                    )
                yield

            def reshard(b, h):
                base = b * N + h * NH
                for j in range(NCORE):
                    nc.sync.dma_start(
                        a2a_in[b][h][j],
                        shard[:, base + WTOK * j:base + WTOK * (j + 1)])
                nc.gpsimd.collective_compute(
                    "AllToAll",
                    mybir.AluOpType.bypass,
                    replica_groups=[list(range(NCORE))],
                    ins=[a2a_in[b][h][:]],
                    outs=[a2a_out[b][h][:]],
                )

            # ---- output projection (one 128-token half-chunk) as filler ----
            wp1 = wpp.tile([128, 8, 512], BF, tag="wpA")
            wp2 = wpp.tile([128, 8, 512], BF, tag="wpB")
            wpr = wpT_d[:].rearrange("(c p) n -> p c n", p=128)
            wps = (wp1, wp2)

            def proj_steps(b, h, tail=False):
                gat = gatp.tile([128, 8, WTOK], BF, tag="gat")
                nc.sync.dma_start(
                    gat[:], a2a_out[b][h][:].rearrange("c p n -> p c n"))
                yield
                for d5 in range(2):
                    if tail and d5 == 1:
                        # attention is over: the scores pool is free, so the
                        # two projection chains can run on separate psum
                        psw = pbig.tile([128, 1024], F32, tag="pbig")
                        ps = psw[:, 0:512]
                    else:
                        ps = psml.tile([128, 512], F32, tag="psml")
                    for c in range(8):
                        nc.tensor.matmul(
                            ps[:],
                            gat[:, c, :],
                            wps[d5][:, c, :],
                            start=(c == 0), stop=(c == 7))
                        if c == 3:
                            yield
                    ysb = yp.tile([128, 512], F32, tag="ysb")
                    nc.vector.tensor_add(
                        ysb[:], ps[:], bpb[:, d5 * 512:(d5 + 1) * 512])
                    row0 = b * 256 + h * 128
                    nc.sync.dma_start(
                        out_d[row0:row0 + 128, d5 * 512:(d5 + 1) * 512],
                        ysb[:])
                    yield

            def batch_steps(b):
                yield from qkv_steps(2 * b)
                yield from norm_steps(2 * b)
                yield from qkv_steps(2 * b + 1)
                yield from norm_steps(2 * b + 1)

            def pro_steps():
                yield from batch_steps(0)

            # ---- pipeline: batch b+1's qkv/norm and batch b-1's
            # out-projection half-chunks are emitted as filler inside batch
            # b's attention; each half-batch reshard overlaps downstream
            # compute. proj(b-1, 1) is padded deep into the filler so its
            # gat load never queues before its collective has landed. ----
            import itertools

            def pad_steps(n):
                for _ in range(n):
                    yield

            # stream the startup loads: first x half-chunk, then wq per
            # c-chunk - the first q matmul starts as soon as c=0's weight
            # slice and the x half it reads have landed
            pre_xt = {}
            xt00 = xtp.tile([128, 8, 512], BF, tag="xt")
            pre_xt[(0, 0)] = xt00
            xr0 = xT_d[:].rearrange("(c p) t -> p c t", p=128)
            nc.sync.dma_start(xt00[:, 0:4, :], xr0[:, 0:4, 0:512])
            for c in range(4):
                nc.sync.dma_start(w_sb[:, c, 0:QKCH], wqr[:, c, 0:QKCH])
            nc.sync.dma_start(xt00[:, 4:8, :], xr0[:, 4:8, 0:512])
            for c in range(4, 8):
                nc.sync.dma_start(w_sb[:, c, 0:QKCH], wqr[:, c, 0:QKCH])
            for c in range(8):
                nc.sync.dma_start(w_sb[:, c, QKCH:2 * QKCH],
                                  wqr[:, c, QKCH:2 * QKCH])
            pro = pro_steps()
            next(pro, None)          # xt(t5=0) queued, qk matmuls out
            nc.sync.dma_start(w_sb[:, :, 2 * QKCH:3 * QKCH],
                              wqr[:, :, 2 * QKCH:3 * QKCH])
            for _ in range(2):
                next(pro, None)
            emit_setup()
            for _ in range(5):
                next(pro, None)
            emit_rope_loads()
            emit_bpb()
            for _ in pro:
                pass
            nc.sync.dma_start(wp1[:], wpr[:, :, 0:512])
            nc.sync.dma_start(wp2[:], wpr[:, :, 512:1024])

            fillers = {}
            for b in range(B):
                parts = []
                nsteps = 0
                if b < B - 1:
                    parts.append(batch_steps(b + 1))
                    nsteps += 28
                if b == B - 1:
                    # rebalance: b3 has no qkv filler (ACT-paced, PE slack),
                    # so it takes one of b2's projection half-chunks
                    parts.append(pad_steps(4))
                    parts.append(proj_steps(1, 1))
                if 1 <= b < B - 1:
                    # proj(b-1, 0) must not hit the queues before its
                    # collective (launched ~1 chunk ago) has landed
                    parts.append(pad_steps(max(0, 18 - nsteps)))
                    parts.append(proj_steps(b - 1, 0))
                    if b == 1:
                        parts.append(pad_steps(max(0, 42 - max(nsteps, 18) - 3)))
                        parts.append(proj_steps(b - 1, 1))
                fillers[b] = itertools.chain(*parts)

            # flat chunk sequence; chunk i's denominator epilogue is emitted
            # inside chunk i+1's first kt slots, and each half-batch reshard
            # fires as soon as both its heads' epilogues have drained
            chunks = [(b, qc, hl)
                      for b in range(B) for qc in range(2) for hl in range(2)]
            prev_epi = None
            for (b, qc, hl) in chunks:
                pv = attention_chunk(hl, b, qc, fillers[b], prev_epi)
                if prev_epi is not None:
                    for _ in prev_epi:
                        pass
                prev_epi = chunk_epilogue(hl, b, qc, pv, do_reshard=(hl == 1))
                if qc == 1 and hl == 1:
                    # batch boundary: the remaining filler (next batch's
                    # qkv/norm tail) must be fully emitted before the next
                    # batch's attention chunks reference it
                    for _ in fillers[b]:
                        pass
            for _ in prev_epi:
                pass
            for _ in proj_steps(B - 2, 0, tail=True):
                pass
            for _ in proj_steps(B - 2, 1, tail=True):
                pass
            for _ in proj_steps(B - 1, 0, tail=True):
                pass
            for _ in proj_steps(B - 1, 1, tail=True):
                pass

    nc.compile()
    _BUILD_CACHE["nc"] = nc
    return nc


def _host_prep(x, rope_cos, rope_sin, w_qkv, w_proj, b_proj, q_norm_w, k_norm_w):
    x = np.asarray(x, np.float32)
    xT = np.ascontiguousarray(x.reshape(T, C).T).astype(BF16)
    cosT = np.asarray(rope_cos, np.float32)[0, 0].T          # [hd, N]
    sinT = np.asarray(rope_sin, np.float32)[0, 0].T

    def fold(w):
        w = np.asarray(w, np.float32)
        cw = (cosT * w[:, None]).astype(BF16)
        sw = np.empty_like(sinT)
        sw[:32] = -sinT[:32] * w[32:64, None]
        sw[32:] = sinT[32:] * w[0:32, None]
        return cw, sw.astype(BF16)

    cosq, sinq = fold(q_norm_w)
    cosk, sink = fold(k_norm_w)
    wpT = np.ascontiguousarray(np.asarray(w_proj, np.float32).T).astype(BF16)
    bp = np.asarray(b_proj, np.float32).reshape(1, C).astype(BF16)
    w_qkv = np.asarray(w_qkv, np.float32)
    consts = np.zeros((2, 130), dtype=BF16)
    consts[0, 0:64] = 1.0
    consts[1, 64:128] = 1.0
    consts[0, 128] = 8.0
    consts[1, 129] = 8.0

    in_maps = []
    for r in range(NCORE):
        wq = w_qkv[QKCH * r:QKCH * (r + 1), :].T
        wk = w_qkv[C + QKCH * r:C + QKCH * (r + 1), :].T
        wv = w_qkv[2 * C + QKCH * r:2 * C + QKCH * (r + 1), :].T
        wqkvT = np.ascontiguousarray(
            np.concatenate([wq, wk, wv], axis=1)).astype(BF16)
        in_maps.append({
            "xT": xT, "wqkvT": wqkvT, "wpT": wpT, "bp": bp,
            "cosq": cosq, "sinq": sinq, "cosk": cosk, "sink": sink,
            "consts": consts,
        })
    return in_maps


def _run(in_maps, trace=False, **kwargs):
    nc = _build()
    return run_bass_kernel_spmd(
        nc, in_maps, core_ids=list(range(NCORE)), trace=trace, **kwargs)


def _gather(res):
    """Core r's out rows are eight 128-token chunks, one per (batch, half):
    global tokens b*N + h*NH + r*WTOK : +WTOK."""
    y = np.empty((T, C), np.float32)
    for r in range(NCORE):
        o = np.asarray(res.results[r]["out"], np.float32)
        for b in range(B):
            for h in range(2):
                dst0 = b * N + h * NH + WTOK * r
                src0 = b * 256 + h * 128
                y[dst0:dst0 + WTOK] = o[src0:src0 + WTOK]
    return y.reshape(B, N, C)


def kernel(**inputs):
    in_maps = _host_prep(**inputs)
    res = _run(in_maps)
    return _gather(res)
